# revision 22
# baseline (speedup 1.0000x reference)
"""Trainium2 Bass kernel for CapsNet dynamic routing (nn_Capsule_13692355740297).

Math (per batch element):
    u_hat[i, (n,d)] = u[i, :] @ W[:, (n,d)]            # never materialized
    iter1: c uniform 1/10  -> s1 = 0.1 * (sum_i u_i)^T W
    iter k: b[i, n] = u_i . v_n   with v_n = W_n o_n   # contract Din on PE
            c = softmax_n(b)                           # free-dim softmax, [i,n] layout
            R[d, n] = sum_i c[i, n] u[i, d]            # contract i on PE
            s[n, :] = sum_d R[d, n] W[d, (n,:)]        # ones-column matmul
            o = squash(s)                              # sqrt via DVE bit-trick rsqrt

Host supplies u in BOTH layouts so no on-chip transposes are needed:
    un[b, p, (j, d)] = u[b, 128j+p, d]   bf16, i on partitions (R-pass stationaries)
    ut[b, d, i]      = u[b, i, d]        fp8e4m3, d on partitions (b-pass stationaries)
The fp8 b-pass only perturbs routing logits (softmax weights); the output path
(R-pass) stays bf16. Verified rel err ~4e-3 vs f32 reference.

Batches are processed in pairs; each pair's chain is emitted in DMA-arrival
("ready") order so the per-pair chains pipeline behind the DMA stream.
Act engine uses only {Exp, Square, Copy} = one table set, zero table reloads.
Sharding: data-parallel over batch, 8 batch elements per core, no collectives.
"""

import numpy as np

B, I_FULL, DIN = 64, 4096, 128
NCAP, DCAP = 10, 16
KND = NCAP * DCAP  # 160
NCORES = 8
BC = B // NCORES  # 8 batch elements per core
NT = I_FULL // 128  # 32 i-tiles per batch
GS = 2  # batches per group
NG = BC // GS
MAGIC = 0x5F3759DF
UT_FP8 = True


def build_nc(bc=BC, nt=NT):
    import concourse.bacc as bacc
    import concourse.mybir as mybir
    from concourse.tile import TileContext

    fp32 = mybir.dt.float32
    bf16 = mybir.dt.bfloat16
    i32 = mybir.dt.int32
    ut_dt = mybir.dt.float8e4 if UT_FP8 else bf16
    AX = mybir.AxisListType
    ALU = mybir.AluOpType
    ACTF = mybir.ActivationFunctionType

    il = nt * 128  # I per batch

    nc = bacc.Bacc(trn_type="TRN2")
    un_h = nc.dram_tensor("un", [bc, 128, il], bf16, kind="ExternalInput")
    ut_h = nc.dram_tensor("ut", [bc, 128, il], ut_dt, kind="ExternalInput")
    w_h = nc.dram_tensor("w", [DIN, KND], fp32, kind="ExternalInput")
    wt_hi_h = nc.dram_tensor("wt_hi", [128, DIN], bf16, kind="ExternalInput")
    wt_lo_h = nc.dram_tensor("wt_lo", [32, DIN], bf16, kind="ExternalInput")
    m_hi_h = nc.dram_tensor("m_hi", [128, NCAP], bf16, kind="ExternalInput")
    m_lo_h = nc.dram_tensor("m_lo", [32, NCAP], bf16, kind="ExternalInput")
    ones_h = nc.dram_tensor("ones", [128, 1], bf16, kind="ExternalInput")
    tenth_h = nc.dram_tensor("tenth", [128, 1], bf16, kind="ExternalInput")
    id1_h = nc.dram_tensor("id1", [1, 1], fp32, kind="ExternalInput")
    out_h = nc.dram_tensor("out", [bc * KND], fp32, kind="ExternalOutput")

    with TileContext(nc) as tc:
        with (
            tc.tile_pool(name="big", bufs=1) as big,
            tc.tile_pool(name="sb", bufs=3) as sb,
            tc.tile_pool(name="sb4", bufs=4) as sb4,
            tc.tile_pool(name="tmpp", bufs=2) as tmpp,
            tc.tile_pool(name="psB", bufs=3, space="PSUM") as psB,
            tc.tile_pool(name="psS", bufs=5, space="PSUM") as psS,
        ):
            nc.scalar.add_instruction(
                mybir.InstLoadActFuncSet(
                    name="I-ldactset0", ins=[], outs=[], act_func_set_id=6
                )
            )
            # ---------- persistent SBUF ----------
            UN = big.tile([128, bc * il], bf16, name="UN_sb")   # [p, (b,j,d)]
            UT = big.tile([128, bc * il], ut_dt, name="UT_sb")  # [d, (b,i)]
            W_sb = big.tile([128, KND], fp32, name="W_sb")
            wt_hi = big.tile([128, DIN], bf16, name="wt_hi_sb")
            wt_lo = big.tile([32, DIN], bf16, name="wt_lo_sb")
            m_hi = big.tile([128, NCAP], bf16, name="m_hi_sb")
            m_lo = big.tile([32, NCAP], bf16, name="m_lo_sb")
            ones = big.tile([128, 1], bf16, name="ones_sb")
            tenth = big.tile([128, 1], bf16, name="tenth_sb")
            id1 = big.tile([1, 1], fp32, name="id1_sb")
            Wb = big.tile([128, KND], bf16, name="Wb_sb")

            nc.gpsimd.dma_start(out=W_sb[:, :], in_=w_h.ap())
            nc.gpsimd.dma_start(out=wt_hi[:, :], in_=wt_hi_h.ap())
            nc.gpsimd.dma_start(out=wt_lo[:, :], in_=wt_lo_h.ap())
            nc.gpsimd.dma_start(out=m_hi[:, :], in_=m_hi_h.ap())
            nc.gpsimd.dma_start(out=m_lo[:, :], in_=m_lo_h.ap())
            nc.gpsimd.dma_start(out=ones[:, :], in_=ones_h.ap())
            nc.gpsimd.dma_start(out=tenth[:, :], in_=tenth_h.ap())
            nc.gpsimd.dma_start(out=id1[:, :], in_=id1_h.ap())

            nc.scalar.copy(out=Wb[:, :], in_=W_sb[:, :])

            UNv = UN[:, :].rearrange("p (b i) -> p b i", b=bc, i=il)
            UNt = UN[:, :].rearrange("p (b j d) -> p b j d", b=bc, j=nt, d=128)
            UTv = UT[:, :].rearrange("p (b i) -> p b i", b=bc, i=il)
            Wv = W_sb[:, :].rearrange("p (n d) -> p n d", n=NCAP)

            dma_order = [("un", 0), ("un", 1), ("un", 2), ("ut", 0),
                         ("un", 3), ("ut", 1), ("un", 4), ("ut", 2),
                         ("un", 5), ("ut", 3), ("un", 6), ("ut", 4),
                         ("un", 7), ("ut", 5), ("ut", 6), ("ut", 7)]
            for kind_, b in dma_order:
                if kind_ == "un":
                    nc.sync.dma_start(out=UNv[:, b], in_=un_h.ap()[b])
                else:
                    nc.sync.dma_start(out=UTv[:, b], in_=ut_h.ap()[b])

            state = [dict() for _ in range(NG)]

            def S0a(b):
                """r0[d] = 0.1 * sum_i u[i, d] for one batch.
                Group 0 runs DVE tree-folds (DVE is idle early); later groups
                use PE-accumulated ones-matmuls over the un tiles."""
                g, col = b // GS, b % GS
                if col == 0:
                    state[g]["sm0"] = psS.tile([128, 512], fp32, name=f"sm0_{g}", tag="sm")
                    state[g]["r0sb"] = sb.tile([128, GS], bf16, name=f"r0sb{g}", tag="r0sb")
                sm0 = state[g]["sm0"]
                if b < 2 * GS:
                    tmp = tmpp.tile([128, 3968], bf16, name=f"tmp{b}", tag="tmp")
                    src = UNv[:, b]
                    nc.vector.tensor_tensor(
                        out=tmp[:, 0:2048], in0=src[:, 0:2048], in1=src[:, 2048:4096],
                        op=ALU.add,
                    )
                    spans = [(0, 2048)]
                    pos = 2048
                    for w in (1024, 512, 256, 128):
                        s0, _ = spans[-1]
                        nc.vector.tensor_tensor(
                            out=tmp[:, pos : pos + w],
                            in0=tmp[:, s0 : s0 + w],
                            in1=tmp[:, s0 + w : s0 + 2 * w],
                            op=ALU.add,
                        )
                        spans.append((pos, w))
                        pos += w
                    nc.tensor.matmul(sm0[:, col : col + 1], tmp[:, 3840:3968], tenth[:, :])
                else:
                    for j in range(nt):
                        nc.tensor.matmul(
                            sm0[:, col : col + 1],
                            UNt[:, b, j],
                            tenth[:, :],
                            start=(j == 0),
                            stop=(j == nt - 1),
                        )
                nc.scalar.copy(
                    out=state[g]["r0sb"][:, col : col + 1], in_=sm0[:, col : col + 1]
                )

            def squash(g, it, s_ps):
                """squash on a [1, GS*KND] PSUM slice -> SBUF [1, GS*KND] f32."""
                NN = GS * NCAP
                sq = sb.tile([1, GS * KND], fp32, name=f"sq{g}_{it}", tag="sq")
                q = sb.tile([1, NN], fp32, name=f"q{g}_{it}", tag="q")
                lnq = sb.tile([1, NN], fp32, name=f"lnq{g}_{it}", tag="lnq")
                sqq = sb.tile([1, NN], fp32, name=f"sqq{g}_{it}", tag="sqq")
                den = sb.tile([1, NN], fp32, name=f"den{g}_{it}", tag="den")
                rden = sb.tile([1, NN], fp32, name=f"rden{g}_{it}", tag="rden")
                coef = sb.tile([1, NN], fp32, name=f"coef{g}_{it}", tag="coef")
                o = sb.tile([1, GS * KND], fp32, name=f"o{g}_{it}", tag="o")
                nc.scalar.activation(sq[:, :], s_ps, ACTF.Square)
                nc.vector.reduce_sum(
                    out=q[:, :],
                    in_=sq[:, :].rearrange("r (n d) -> r n d", n=GS * NCAP),
                    axis=AX.X, op=ALU.add,
                )
                # sqrt(q) = exp(0.5 ln q); table set 6 holds both exp and ln
                nc.scalar.activation(lnq[:, :], q[:, :], ACTF.Ln)
                nc.scalar.activation(sqq[:, :], lnq[:, :], ACTF.Exp, scale=0.5)
                nc.vector.tensor_scalar_add(den[:, :], q[:, :], 1.0)
                nc.vector.reciprocal(out=rden[:, :], in_=den[:, :])
                nc.vector.tensor_tensor(out=coef[:, :], in0=sqq[:, :], in1=rden[:, :], op=ALU.mult)
                nc.vector.tensor_tensor(
                    out=o[:, :].rearrange("r (n d) -> r n d", n=GS * NCAP),
                    in0=s_ps.rearrange("r (n d) -> r n d", n=GS * NCAP),
                    in1=coef[:, :].unsqueeze(2).broadcast_to([1, GS * NCAP, DCAP]),
                    op=ALU.mult,
                )
                return o

            def make_V(g, it, o, sm):
                """V[d, (b,n)] = sum_e W[d,(n,e)] o[b,n,e]; bf16 [128, GS*NCAP].
                Uses columns 320:344 of the group's rotating PSUM bank tile."""
                for col in range(GS):
                    nc.tensor.transpose(
                        sm[:, 340 + col : 341 + col],
                        o[:, KND * col : KND * col + 128], id1[:, :],
                    )
                    nc.tensor.transpose(
                        sm[:32, 344 + col : 345 + col],
                        o[:, KND * col + 128 : KND * (col + 1)], id1[:, :],
                    )
                oeh = sb.tile([128, GS * NCAP], bf16, name=f"oeh{g}_{it}", tag="oeh")
                oel = sb.tile([32, GS * NCAP], bf16, name=f"oel{g}_{it}", tag="oel")
                nc.vector.tensor_tensor(
                    out=oeh[:, :].rearrange("p (b n) -> p b n", b=GS),
                    in0=sm[:, 340 : 340 + GS].unsqueeze(2).broadcast_to([128, GS, NCAP]),
                    in1=m_hi[:, :].unsqueeze(1).broadcast_to([128, GS, NCAP]),
                    op=ALU.mult,
                )
                nc.vector.tensor_tensor(
                    out=oel[:, :].rearrange("p (b n) -> p b n", b=GS),
                    in0=sm[:32, 344 : 344 + GS].unsqueeze(2).broadcast_to([32, GS, NCAP]),
                    in1=m_lo[:, :].unsqueeze(1).broadcast_to([32, GS, NCAP]),
                    op=ALU.mult,
                )
                vp = sm[:, 320 : 320 + GS * NCAP]
                nc.tensor.matmul(vp, wt_hi[:, :], oeh[:, :], start=True, stop=False)
                nc.tensor.matmul(vp, wt_lo[:, :], oel[:, :], start=False, stop=True)
                V = sb.tile([128, GS * NCAP], bf16, name=f"V{g}_{it}", tag="V")
                nc.scalar.copy(out=V[:, :], in_=vp)
                return V

            def S0b(g):
                sm0 = state[g]["sm0"]
                s1p = sm0[0:1, 24 : 24 + GS * KND]
                for col in range(GS):
                    nc.tensor.matmul(
                        sm0[0:1, 24 + KND * col : 24 + KND * (col + 1)],
                        state[g]["r0sb"][:, col : col + 1],
                        Wb[:, :],
                    )
                o = squash(g, 1, s1p)
                state[g]["V"] = make_V(g, 1, o, sm0)

            def bpass(g, it):
                """b-pass + softmax, in halves; R-pass half fires as soon as
                its cc half exists. Leaves the group's sm tile in state."""
                V = state[g]["V"]
                sm = psS.tile([128, 512], fp32, name=f"sm{g}_{it}", tag="sm")
                state[g]["smR"] = sm
                hh = nt // 2
                ccs = []
                for col in range(GS):
                    b = g * GS + col
                    btp = psB.tile([128, nt * NCAP], fp32, name=f"btp{b}_{it}", tag="btp")
                    eb = sb4.tile([128, nt * NCAP], bf16, name=f"eb{b}_{it}", tag="eb")
                    Z = sb4.tile([128, nt], fp32, name=f"Z{b}_{it}", tag="Z")
                    rZ = sb4.tile([128, nt], fp32, name=f"rZ{b}_{it}", tag="rZ")
                    cc = sb4.tile([128, nt * NCAP], bf16, name=f"cc{b}_{it}", tag="cc")
                    for half in range(2):
                        j0 = half * hh
                        for j in range(j0, j0 + hh):
                            nc.tensor.matmul(
                                btp[:, NCAP * j : NCAP * (j + 1)],
                                UTv[:, b, 128 * j : 128 * (j + 1)],
                                V[:, NCAP * col : NCAP * (col + 1)],
                            )
                        c0, c1 = NCAP * j0, NCAP * (j0 + hh)
                        nc.scalar.activation(eb[:, c0:c1], btp[:, c0:c1], ACTF.Exp)
                        ebv = eb[:, c0:c1].rearrange("p (j n) -> p j n", j=hh)
                        nc.vector.reduce_sum(
                            out=Z[:, j0 : j0 + hh], in_=ebv, axis=AX.X, op=ALU.add
                        )
                        nc.vector.reciprocal(
                            out=rZ[:, j0 : j0 + hh], in_=Z[:, j0 : j0 + hh]
                        )
                        nc.gpsimd.tensor_tensor(
                            out=cc[:, c0:c1].rearrange("p (j n) -> p j n", j=hh),
                            in0=ebv,
                            in1=rZ[:, j0 : j0 + hh].unsqueeze(2)
                            .broadcast_to([128, hh, NCAP]),
                            op=ALU.mult,
                        )
                        for j in range(j0, j0 + hh):
                            nc.tensor.matmul(
                                sm[:, NCAP * col : NCAP * (col + 1)],
                                UNt[:, b, j],
                                cc[:, NCAP * j : NCAP * (j + 1)],
                                start=(j == 0),
                                stop=(j == nt - 1),
                            )
                    ccs.append(cc)

            def rpass(g, it):
                sm = state[g]["smR"]
                prod = sb.tile([128, GS * KND], bf16, name=f"prod{g}_{it}", tag="prod")
                nc.vector.tensor_tensor(
                    out=prod[:, :].rearrange("p (b n d) -> p b n d", b=GS, n=NCAP),
                    in0=sm[:, 0 : GS * NCAP].rearrange("p (b n) -> p b n", b=GS)
                    .unsqueeze(3).broadcast_to([128, GS, NCAP, DCAP]),
                    in1=Wv.unsqueeze(1).broadcast_to([128, GS, NCAP, DCAP]),
                    op=ALU.mult,
                )
                sp = sm[0:1, 24 : 24 + GS * KND]
                for col in range(GS):
                    nc.tensor.matmul(
                        sm[0:1, 24 + KND * col : 24 + KND * (col + 1)],
                        ones[:, :],
                        prod[:, KND * col : KND * (col + 1)],
                    )
                o = squash(g, it, sp)
                if it == 2:
                    state[g]["V"] = make_V(g, it, o, sm)
                else:
                    nc.sync.dma_start(
                        out=out_h.ap()[g * GS * KND : (g + 1) * GS * KND], in_=o[:, :]
                    )

            # ---------- ready-order wavefront emit ----------
            # un arrival estimates under the reordered DMA stream (us)
            un_arr = [12.0, 15.0, 18.0, 22.4, 26.9, 31.3, 35.8, 40.2]
            ut_arr = [19.5, 23.9, 28.4, 32.8, 37.3, 41.7, 43.2, 44.7]
            events = []
            for b in range(bc):
                events.append((un_arr[b] + 0.1, 0, b, "a"))
            for g in range(NG):
                v1 = un_arr[2 * g + 1] + 2.5
                p2 = max(v1 + 4.5, ut_arr[2 * g + 1] + 0.1)
                events.append((v1, 1, g, "b"))
                events.append((p2, 2, g, "p2"))
                events.append((p2 + 4, 3, g, "r2"))
                events.append((p2 + 9, 4, g, "p3"))
                events.append((p2 + 12, 5, g, "r3"))
            events.sort(key=lambda e: (e[0], e[1]))
            for _, _, idx, kind in events:
                if kind == "a":
                    S0a(idx)
                elif kind == "b":
                    S0b(idx)
                elif kind == "p2":
                    bpass(idx, 2)
                elif kind == "r2":
                    rpass(idx, 2)
                elif kind == "p3":
                    bpass(idx, 3)
                else:
                    rpass(idx, 3)

    nc.compile()
    return nc


def make_const_inputs():
    import ml_dtypes

    mask = np.zeros((KND, NCAP), dtype=np.float32)
    for k in range(KND):
        mask[k, k // DCAP] = 1.0
    return {
        "m_hi": mask[:128].astype(ml_dtypes.bfloat16),
        "m_lo": mask[128:].astype(ml_dtypes.bfloat16),
        "ones": np.ones((128, 1), dtype=ml_dtypes.bfloat16),
        "tenth": np.full((128, 1), 0.1, dtype=ml_dtypes.bfloat16),
        "id1": np.ones((1, 1), dtype=np.float32),
    }


def make_w_inputs(W):
    import ml_dtypes

    W = np.asarray(W, dtype=np.float32)
    WT = W.T.copy()  # [160, 128]
    return {
        "w": W,
        "wt_hi": WT[:128].astype(ml_dtypes.bfloat16),
        "wt_lo": WT[128:].astype(ml_dtypes.bfloat16),
    }


def make_u_inputs(u_core):
    """u_core: [BC, 4096, 128] f32 -> un/ut host layouts."""
    import ml_dtypes

    ut_np = ml_dtypes.float8_e4m3 if UT_FP8 else ml_dtypes.bfloat16
    # un[b, p, (j, d)] = u[b, 128j + p, d]
    un = np.ascontiguousarray(
        u_core.reshape(BC, NT, 128, DIN).transpose(0, 2, 1, 3).reshape(BC, 128, NT * DIN)
    ).astype(ml_dtypes.bfloat16)
    # ut[b, d, i] = u[b, i, d]
    ut = np.ascontiguousarray(u_core.transpose(0, 2, 1)).astype(ut_np)
    return {"un": un, "ut": ut}


_CACHE = {}


def kernel(u_vecs, W):
    from concourse import bass_utils

    u_vecs = np.asarray(u_vecs, dtype=np.float32)
    W = np.asarray(W, dtype=np.float32)
    if "nc" not in _CACHE:
        _CACHE["nc"] = build_nc()
    nc = _CACHE["nc"]

    consts = make_const_inputs()
    wis = make_w_inputs(W)
    in_maps = []
    for c in range(NCORES):
        m = dict(make_u_inputs(u_vecs[c * BC : (c + 1) * BC]))
        m.update(consts)
        m.update(wis)
        in_maps.append(m)

    res = bass_utils.run_bass_kernel_spmd(nc, in_maps, core_ids=list(range(NCORES)))
    outs = [r["out"] for r in res.results]
    return np.concatenate(outs, axis=0).reshape(B, NCAP, DCAP).astype(np.float32)
